# revision 20
# baseline (speedup 1.0000x reference)
"""MoE (top-2 of 8 experts + shared expert) Trainium2 Bass kernel.

Strategy (expert-parallel, host-prepped routing, bf16 compute):
  - Router (sigmoid gate + top-2) runs on the host in fp32; it produces the
    token->expert gather lists.
  - Core c computes expert c's SwiGLU FFN densely over the tokens routed to
    it (padded to the max per-expert count), plus the shared-expert FFN over
    the token shard [c*1024, (c+1)*1024).
  - All matmuls are bf16 with 1024-wide moving operands (2-bank PSUM tiles)
    to amortize per-MM issue/LDWEIGHTS overhead; rel err ~4e-3 end to end.
  - The shared pass runs FIRST, streaming its weight slabs on the SP DMA
    ring, while the expert w1/w3 (11.5 MB bf16) preload into SBUF-resident
    tiles on the ACT DMA ring.  w2 slabs are streamed per chunk in both
    passes (SBUF does not fit all three resident at 1024-token chunks).
  - Host scatter-adds the per-expert outputs (scaled by combine weights)
    and the shared outputs into the final [8192, 2048] f32 result.

Everything on-device is feature-major ("K on partitions") so the x @ W.T
chains need no on-chip transposes:
  stage1:  h1T[m,:] = sum_k w1T[k, m].T @ xT[k, :]   (PSUM accum over k)
  g = silu(h1T) * h3T                                 (ACT + DVE, bf16 out)
  stage2:  yT[md,:] = sum_kh w2T[kh, md].T @ gT[kh,:]
"""

import os
import sys

for _p in ("/opt/trn_rl_repo", "/root/.axon_site/_ro/trn_rl_repo"):
    if os.path.isdir(_p) and _p not in sys.path:
        sys.path.insert(0, _p)

import numpy as np
import ml_dtypes

import concourse.bass as bass  # noqa: F401
import concourse.mybir as mybir
import concourse.tile as tile
from concourse import bacc
from concourse.bass_utils import run_bass_kernel_spmd

# Problem constants (hardcoded per spec)
N_TOK = 8192
D = 2048
H = 1408
E = 8
TOP_K = 2
ROUTE_SCALE = 1.0
P = 128
KD = D // P    # 16 k-tiles over D
MH = H // P    # 11 m-tiles over H
MD = D // P    # 16 m-tiles over D (stage 2 out)
SHARD = N_TOK // E  # 1024 shared-expert tokens per core

MAX_CHUNK = int(os.environ.get("MOE_MAX_CHUNK", "1024"))

F32 = mybir.dt.float32
BF16 = mybir.dt.bfloat16
NP_BF16 = ml_dtypes.bfloat16
SILU = mybir.ActivationFunctionType.Silu

LAST_RESULTS = None  # BassKernelResults of the most recent run (for test.py)

SKIP_MM = bool(os.environ.get("MOE_SKIP_MM"))
SKIP_DMA = bool(os.environ.get("MOE_SKIP_DMA"))
FIXED_W = bool(os.environ.get("MOE_FIXED_W"))  # timing probe: one lhsT for all MMs


def _enable_ldw_opt():
    """Turn the neuronxcc LDWEIGHTS optimization back on for our compiles.

    The environment's default flags carry --enable-ldw-opt=false inside
    --internal-backend-options; every bf16 matmul then pays a serialized
    ~53 ns weight load.  concourse.compiler_utils exposes the supported
    flag-override API; we rewrite just that one option.
    """
    if not os.environ.get("MOE_LDW_OPT"):
        return
    try:
        from concourse import compiler_utils
        flags = compiler_utils.get_compiler_flags()
        new = [f.replace("--enable-ldw-opt=false", "--enable-ldw-opt=true")
               for f in flags]
        if new != flags:
            compiler_utils.set_compiler_flags(new)
    except Exception:
        pass


_enable_ldw_opt()


def _chunks(T):
    """Split T (multiple of 128) into greedy chunks of <=MAX_CHUNK.

    Greedy (not balanced) minimizes the total matmul-instruction count:
    full-size chunks get full 512-col subs, and per-MM issue+LDWEIGHTS
    overhead dominates small-N matmuls.
    """
    sizes = [MAX_CHUNK] * (T // MAX_CHUNK)
    if T % MAX_CHUNK:
        sizes.append(T % MAX_CHUNK)
    assert sum(sizes) == T and all(s % 128 == 0 for s in sizes), sizes
    return sizes


def _subs(Tc):
    """Split Tc into matmul free-dim slices of <=512."""
    out = []
    rem = Tc
    while rem > 512:
        take = 384 if rem == 640 else 512
        out.append(take)
        rem -= take
    if rem:
        out.append(rem)
    s0 = 0
    res = []
    for s in out:
        res.append((s0, s))
        s0 += s
    return res


def _emit_ffn(nc, pools, x_dram, y_dram, T, get_w13, get_w2):
    """Emit one feature-major SwiGLU FFN over T tokens.

    get_w13(m) -> (w1_ap, w3_ap) each [P, KD*P]; get_w2(md) -> [P, MH*P].
    """
    xpool, gpool, spool, ypool, psum = pools

    fixed_w = [None]

    def lhs(ap):
        if not FIXED_W:
            return ap
        if fixed_w[0] is None:
            fixed_w[0] = ap
        return fixed_w[0]

    cs = 0
    for Tc in _chunks(T):
        # chunk of x on the ACT DMA ring (does not queue behind SP-ring
        # y writebacks), split into 4 k-group pieces so the first matmul
        # can start after ~1/4 of the transfer
        xt = xpool.tile([P, KD * Tc], BF16, name="xt")
        for k0 in range(0, KD, 4):
            SKIP_DMA or nc.scalar.dma_start(
                xt[:, k0 * Tc:(k0 + 4) * Tc].rearrange("p (k t) -> p k t", k=4),
                x_dram[k0:k0 + 4, :, cs:cs + Tc].rearrange("k p t -> p k t"),
            )
        x_tiles = [xt[:, k * Tc:(k + 1) * Tc] for k in range(KD)]
        subs = _subs(Tc)
        g_tiles = []
        for m in range(MH):
            w1m, w3m = get_w13(m)
            gm = gpool.tile([P, Tc], BF16, name=f"g{m}")
            ps1 = [psum.tile([P, 512], F32, name="acc")[:, :sl] for _, sl in subs]
            ps3 = [psum.tile([P, 512], F32, name="acc")[:, :sl] for _, sl in subs]
            for k in range(KD):
                # consecutive MMs share one lhsT so the PE skips the
                # per-MM weight reload
                w1k = lhs(w1m[:, k * P:(k + 1) * P])
                for j, (s0, sl) in enumerate(subs):
                    SKIP_MM or nc.tensor.matmul(
                        ps1[j], w1k, x_tiles[k][:, s0:s0 + sl],
                        start=(k == 0), stop=(k == KD - 1),
                    )
                w3k = lhs(w3m[:, k * P:(k + 1) * P])
                for j, (s0, sl) in enumerate(subs):
                    SKIP_MM or nc.tensor.matmul(
                        ps3[j], w3k, x_tiles[k][:, s0:s0 + sl],
                        start=(k == 0), stop=(k == KD - 1),
                    )
            for j, (s0, sl) in enumerate(subs):
                st = spool.tile([P, 512], BF16, name="silu")[:, :sl]
                SKIP_MM or nc.scalar.activation(st, ps1[j], SILU)
                SKIP_MM or nc.vector.tensor_mul(gm[:, s0:s0 + sl], st, ps3[j])
            g_tiles.append(gm)
        for md in range(MD):
            w2m = get_w2(md)
            ym = ypool.tile([P, Tc], BF16, name="ym")
            psy = [psum.tile([P, 512], F32, name="acc")[:, :sl] for _, sl in subs]
            for kh in range(MH):
                w2k = lhs(w2m[:, kh * P:(kh + 1) * P])
                for j, (s0, sl) in enumerate(subs):
                    SKIP_MM or nc.tensor.matmul(
                        psy[j], w2k, g_tiles[kh][:, s0:s0 + sl],
                        start=(kh == 0), stop=(kh == MH - 1),
                    )
            for j, (s0, sl) in enumerate(subs):
                SKIP_MM or nc.vector.tensor_copy(ym[:, s0:s0 + sl], psy[j])
            SKIP_MM or nc.sync.dma_start(y_dram[md, :, cs:cs + Tc], ym[:])
        cs += Tc


def _build_program(c_cap, loop_reps=1):
    nc = bacc.Bacc("TRN2", target_bir_lowering=False, debug=False, num_devices=E)
    xe = nc.dram_tensor("xe", [KD, P, c_cap], BF16, kind="ExternalInput").ap()
    xs = nc.dram_tensor("xs", [KD, P, SHARD], BF16, kind="ExternalInput").ap()
    # resident expert w1/w3, partition-major (one big DMA each)
    w1r = nc.dram_tensor("w1r", [P, MH * KD * P], BF16, kind="ExternalInput").ap()
    w3r = nc.dram_tensor("w3r", [P, MH * KD * P], BF16, kind="ExternalInput").ap()
    # streamed slabs: expert w2 + all shared weights
    w2s = nc.dram_tensor("w2s", [MD, P, MH * P], BF16, kind="ExternalInput").ap()
    sw1s = nc.dram_tensor("sw1s", [MH, P, KD * P], BF16, kind="ExternalInput").ap()
    sw3s = nc.dram_tensor("sw3s", [MH, P, KD * P], BF16, kind="ExternalInput").ap()
    sw2s = nc.dram_tensor("sw2s", [MD, P, MH * P], BF16, kind="ExternalInput").ap()
    ye = nc.dram_tensor("ye", [MD, P, c_cap], BF16, kind="ExternalOutput").ap()
    ys = nc.dram_tensor("ys", [MD, P, SHARD], BF16, kind="ExternalOutput").ap()

    with tile.TileContext(nc) as tc:
        with tc.tile_pool(name="res", bufs=1) as res, \
             tc.tile_pool(name="xpool", bufs=1) as xpool, \
             tc.tile_pool(name="wpool", bufs=3) as wpool, \
             tc.tile_pool(name="w2pool", bufs=4) as w2pool, \
             tc.tile_pool(name="gpool", bufs=1) as gpool, \
             tc.tile_pool(name="spool", bufs=3) as spool, \
             tc.tile_pool(name="ypool", bufs=3) as ypool, \
             tc.tile_pool(name="psum", bufs=8, space="PSUM") as psum:
            pools = (xpool, gpool, spool, ypool, psum)

            def body():
                w1t = res.tile([P, MH * KD * P], BF16, name="w1t")
                w3t = res.tile([P, MH * KD * P], BF16, name="w3t")

                def stream_w13(m):
                    w1m = wpool.tile([P, KD * P], BF16, name="w1m")
                    SKIP_DMA or nc.sync.dma_start(w1m[:], sw1s[m])
                    w3m = wpool.tile([P, KD * P], BF16, name="w3m")
                    SKIP_DMA or nc.sync.dma_start(w3m[:], sw3s[m])
                    return w1m[:], w3m[:]

                def res_w13(m):
                    o = m * KD * P
                    return w1t[:, o:o + KD * P], w3t[:, o:o + KD * P]

                def mk_w2(dram):
                    def get(md):
                        w2m = w2pool.tile([P, MH * P], BF16, name="w2m")
                        SKIP_DMA or nc.sync.dma_start(w2m[:], dram[md])
                        return w2m[:]
                    return get

                _emit_ffn(nc, pools, xs, ys, SHARD, stream_w13, mk_w2(sw2s))
                # resident expert w1/w3: preload on the ACT DMA ring, emitted
                # after the shared pass so its x DMAs go first in ring order;
                # the transfer overlaps the shared pass's compute
                if not SKIP_DMA:
                    nc.scalar.dma_start(w1t[:], w1r)
                    nc.scalar.dma_start(w3t[:], w3r)
                _emit_ffn(nc, pools, xe, ye, c_cap, res_w13, mk_w2(w2s))

            if loop_reps > 1:
                with tc.For_i(0, loop_reps, 1):
                    body()
            else:
                body()
    nc.compile()
    return nc


def _tile_w13_stream(w):
    # [H, D] -> [MH, P, KD*P] with slab[m, p, k*P+j] = w[m*P+j, k*P+p]
    return np.ascontiguousarray(
        w.reshape(MH, P, KD, P).transpose(0, 3, 2, 1).reshape(MH, P, KD * P)
    )


def _tile_w2_stream(w):
    # [D, H] -> [MD, P, MH*P] with slab[md, p, kh*P+j] = w[md*P+j, kh*P+p]
    return np.ascontiguousarray(
        w.reshape(MD, P, MH, P).transpose(0, 3, 2, 1).reshape(MD, P, MH * P)
    )


def _tile_w13_res(w):
    # [H, D] -> [P, MH*KD*P] with t[p, (m*KD+k)*P+j] = w[m*P+j, k*P+p]
    return np.ascontiguousarray(
        w.reshape(MH, P, KD, P).transpose(3, 0, 2, 1).reshape(P, MH * KD * P)
    )


def _tile_x(xt):
    # [T, D] -> [KD, P, T]
    T = xt.shape[0]
    return np.ascontiguousarray(xt.reshape(T, KD, P).transpose(1, 2, 0))


def _untile_y(y):
    # [MD, P, T] -> [T, D]
    return y.transpose(2, 0, 1).reshape(y.shape[2], D).astype(np.float32)


def prepare(x, gate_w, expert_bias, w1, w2, w3, sw1, sw2, sw3):
    """Host routing + input prep. Returns (nc, in_maps, meta)."""
    x = np.ascontiguousarray(np.asarray(x, dtype=np.float32))
    gate_w = np.asarray(gate_w, dtype=np.float32)
    expert_bias = np.asarray(expert_bias, dtype=np.float32)
    w1 = np.asarray(w1, dtype=np.float32)
    w2 = np.asarray(w2, dtype=np.float32)
    w3 = np.asarray(w3, dtype=np.float32)
    sw1 = np.asarray(sw1, dtype=np.float32)
    sw2 = np.asarray(sw2, dtype=np.float32)
    sw3 = np.asarray(sw3, dtype=np.float32)

    # ---- host router (fp32, matches reference numerics) ----
    logits = x @ gate_w.T  # [N, E] f32
    scores = np.where(
        logits >= 0,
        1.0 / (1.0 + np.exp(-logits, dtype=np.float32)),
        np.exp(logits, dtype=np.float32) / (1.0 + np.exp(logits, dtype=np.float32)),
    ).astype(np.float32)
    biased = scores + expert_bias[None, :]
    i1 = np.argmax(biased, axis=1)
    tmp = biased.copy()
    tmp[np.arange(N_TOK), i1] = -np.inf
    i2 = np.argmax(tmp, axis=1)
    s1 = scores[np.arange(N_TOK), i1]
    s2 = scores[np.arange(N_TOK), i2]
    denom = s1 + s2 + np.float32(1e-20)
    c1 = (s1 / denom * np.float32(ROUTE_SCALE)).astype(np.float32)
    c2 = (s2 / denom * np.float32(ROUTE_SCALE)).astype(np.float32)

    idx_list, cw_list = [], []
    for e in range(E):
        m1 = i1 == e
        m2 = i2 == e
        idx = np.concatenate([np.nonzero(m1)[0], np.nonzero(m2)[0]])
        cw = np.concatenate([c1[m1], c2[m2]]).astype(np.float32)
        idx_list.append(idx)
        cw_list.append(cw)
    counts = [len(i) for i in idx_list]
    c_cap = max(512, -(-max(counts) // 128) * 128)

    # ---- build + compile the SPMD program for this capacity ----
    nc = _build_program(c_cap, loop_reps=int(os.environ.get("MOE_LOOP_REPS", "1")))

    # ---- per-core inputs ----
    x_bf = x.astype(NP_BF16)
    in_maps = []
    sw1s = _tile_w13_stream(sw1.astype(NP_BF16))
    sw3s = _tile_w13_stream(sw3.astype(NP_BF16))
    sw2s = _tile_w2_stream(sw2.astype(NP_BF16))
    for c in range(E):
        idx = idx_list[c]
        pad = c_cap - len(idx)
        idx_pad = np.concatenate([idx, np.zeros(pad, dtype=idx.dtype)]) if pad else idx
        in_maps.append({
            "xe": _tile_x(x_bf[idx_pad]),
            "xs": _tile_x(x_bf[c * SHARD:(c + 1) * SHARD]),
            "w1r": _tile_w13_res(w1[c].astype(NP_BF16)),
            "w3r": _tile_w13_res(w3[c].astype(NP_BF16)),
            "w2s": _tile_w2_stream(w2[c].astype(NP_BF16)),
            "sw1s": sw1s,
            "sw3s": sw3s,
            "sw2s": sw2s,
        })

    meta = (idx_list, cw_list, counts)
    return nc, in_maps, meta


def combine(meta, results):
    """Scatter-add per-core outputs into the final [N, D] array."""
    idx_list, cw_list, counts = meta
    out = np.zeros((N_TOK, D), dtype=np.float32)
    for c in range(E):
        r = results[c]
        cnt = counts[c]
        if cnt:
            y_tok = _untile_y(r["ye"])[:cnt]
            out[idx_list[c]] += cw_list[c][:, None] * y_tok
        out[c * SHARD:(c + 1) * SHARD] += _untile_y(r["ys"])
    return out


def kernel(x, gate_w, expert_bias, w1, w2, w3, sw1, sw2, sw3):
    nc, in_maps, meta = prepare(x, gate_w, expert_bias, w1, w2, w3, sw1, sw2, sw3)
    global LAST_RESULTS
    res = run_bass_kernel_spmd(nc, in_maps, core_ids=list(range(E)))
    LAST_RESULTS = res
    return combine(meta, res.results)
